# revision 1
# baseline (speedup 1.0000x reference)
"""AdaptiveGroupNorm (global mean/var over the whole tensor) on 8 TRN2 cores.

reference semantics (indexes == arange(N*C), so the gather/scatter is identity):
    mean = x.mean();  var = ((x - mean)**2).sum() / (x.size - 1)
    out  = (x - mean) / sqrt(var + eps) * weight + bias     (weight/bias per-channel)

Strategy: data-parallel over N (4 batches per core, 16 MiB/core kept fully in
SBUF).  Local Σx / Σx² are computed per-tile while the load DMAs stream in,
folded across partitions with a ones-vector matmul and all-reduced (32 B)
across the 8 cores, then each tile is normalized in place and stored.  HBM traffic per core is exactly one read + one write of the shard.
"""

import numpy as np

import concourse.bass as bass
import concourse.bacc as bacc
import concourse.tile as tile
from concourse import mybir
from concourse import bass2jax

N_CORES = 8
EPS = 1e-5
N, C, H, W = 32, 256, 64, 64
N_LOC = N // N_CORES            # 4 batches per core
ROWS = N_LOC * C                # 1024 (n,c) rows per core
F = H * W                       # 4096 elements per row
P = 128                         # partitions
NTILES = ROWS // P              # 8 logical row-tiles of (128, 4096)
CNT = N * C * H * W             # global element count
FP32 = mybir.dt.float32

# load/compute chunks: (row_tile_start, n_row_tiles, col_start, col_len).
# 2 MiB transfers up front for DMA efficiency, then halves and quarters so
# the final chunk's stats land sooner after its load completes (the
# AllReduce triggers off the last chunk) and the first normalize+store
# launches sooner after the stats broadcast.
CHUNKS = ([(t, 1, 0, F) for t in range(4)]
          + [(t, 1, c, F // 2) for t in range(4, 6) for c in (0, F // 2)]
          + [(t, 1, c, F // 4) for t in range(6, 8) for c in (0, F // 4, F // 2, 3 * F // 4)])
NCH = len(CHUNKS)


def build_nc(affine: bool = True) -> bass.Bass:
    """affine=False specializes weight==1, bias==0 (the spec's fills):
    A = rstd and B = -mean*rstd for every channel, dropping the per-channel
    coefficient ops from the post-allreduce critical path."""
    nc = bacc.Bacc("TRN2", target_bir_lowering=False, debug=False, num_devices=N_CORES)

    x_ext = nc.declare_dram_parameter("x", [N_LOC, C, H, W], FP32, isOutput=False)
    if affine:
        w_ext = nc.declare_dram_parameter("weight", [1, C, 1, 1], FP32, isOutput=False)
        b_ext = nc.declare_dram_parameter("bias", [1, C, 1, 1], FP32, isOutput=False)
    out_ext = nc.declare_dram_parameter("out", [N_LOC, C, H, W], FP32, isOutput=True)

    # (p, t, f) views: row r = t*128 + p maps to channel (r % 256), so even
    # row-tiles hold channels 0..127 and odd row-tiles channels 128..255.
    xv = x_ext.ap().rearrange("n c h w -> (n c) (h w)").rearrange("(t p) f -> p t f", p=P)
    ov = out_ext.ap().rearrange("n c h w -> (n c) (h w)").rearrange("(t p) f -> p t f", p=P)
    if affine:
        # weight/bias as (128, 2): col 0 = ch 0..127, col 1 = ch 128..255
        wv = w_ext.ap().rearrange("a c b d -> (a b d c)").rearrange("(t p) -> p t", p=P)
        bv = b_ext.ap().rearrange("a c b d -> (a b d c)").rearrange("(t p) -> p t", p=P)

    with tile.TileContext(nc, num_cores=N_CORES) as tc:
        with (
            tc.tile_pool(name="data", bufs=1) as data,
            tc.tile_pool(name="scratch", bufs=2) as scratch,
            tc.tile_pool(name="small", bufs=1) as small,
            tc.tile_pool(name="psum", bufs=2, space="PSUM") as psum,
            tc.tile_pool(name="dram", bufs=1, space="DRAM") as dram,
        ):
            ones_t = small.tile([P, 1], FP32)
            nc.vector.memset(ones_t, 1.0)
            eps_t = small.tile([P, 1], FP32)
            nc.vector.memset(eps_t, EPS)
            # scalar (ACT) HWDGE ring: keep the sync FIFO free so the first
            # big x-load issues immediately
            if affine:
                w_t = small.tile([P, 2], FP32)
                b_t = small.tile([P, 2], FP32)
                nc.scalar.dma_start(out=w_t, in_=wv)
                nc.scalar.dma_start(out=b_t, in_=bv)


            # cols 0..NCH-1 = per-chunk Σx, NCH..2*NCH-1 = per-chunk Σx²
            parts = small.tile([P, 2 * NCH], FP32)
            # hoisted: cc staging buffer zeroed in the preamble so the
            # zeroing never sits on the stats critical path
            cc_sbs = {}
            for gname in ("a",):
                cc_sb = small.tile([1, 8], FP32, tag=f"ccsb_{gname}")
                nc.vector.memset(cc_sb, 0.0)
                cc_sbs[gname] = cc_sb

            def stats_exchange(gname, col0, k):
                """Fold this group's partials across partitions and kick off
                its 32 B AllReduce.  All small DMAs ride the gpsimd (SWDGE)
                ring: both HWDGE rings are busy mid-load (sync with the
                16 MiB of x loads, scalar with the Square pass) and their
                FIFO order would delay the collective trigger by ~25 µs.
                Interleaved emission also matters: these DVE/PE ops must
                precede the remaining chunks' ops in each engine's static
                program order, or the engines block on later loads first."""
                ps = psum.tile([1, 2 * k], FP32, tag=f"fold_{gname}")
                nc.tensor.matmul(
                    ps, ones_t, parts[:, col0 : col0 + 2 * k], start=True, stop=True
                )
                cc_sb = cc_sbs[gname]
                nc.vector.reduce_sum(
                    out=cc_sb[:, 0:2],
                    in_=ps.rearrange("p (g k) -> p g k", g=2),
                    axis=mybir.AxisListType.X,
                )
                cc_in = dram.tile([1, 8], FP32, tag=f"ccin_{gname}")
                cc_out = dram.tile([1, 8], FP32, tag=f"ccout_{gname}")
                nc.gpsimd.dma_start(out=cc_in[:], in_=cc_sb)
                nc.gpsimd.collective_compute(
                    "AllReduce",
                    mybir.AluOpType.add,
                    replica_groups=[list(range(N_CORES))],
                    ins=[cc_in.opt()],
                    outs=[cc_out.opt()],
                )
                return cc_out

            chunk_tiles = []
            cc_outs = []
            for ci, (t0, nt, c0, clen) in enumerate(CHUNKS):
                xt = data.tile([P, nt, clen], FP32, tag=f"xt{ci}")
                nc.sync.dma_start(out=xt, in_=xv[:, t0 : t0 + nt, c0 : c0 + clen])
                chunk_tiles.append(xt)
                cs = ci
                cq = NCH + ci
                nc.vector.reduce_sum(
                    out=parts[:, cs : cs + 1], in_=xt, axis=mybir.AxisListType.XY
                )
                sq = scratch.tile([P, nt * clen], FP32, tag="sq")
                nc.scalar.activation(
                    out=sq[:, : nt * clen],
                    in_=xt.rearrange("p t f -> p (t f)"),
                    func=mybir.ActivationFunctionType.Square,
                    accum_out=parts[:, cq : cq + 1],
                )
            cc_outs.append(stats_exchange("a", 0, NCH))

            cc_ap = cc_outs[0][:]
            stats = small.tile([P, 2], FP32)
            bc_src = bass.AP(
                tensor=cc_ap.tensor, offset=cc_ap.offset, ap=[[0, P], [1, 2]]
            )
            nc.scalar.dma_start(out=stats, in_=bc_src)
            S = stats[:, 0:1]
            SS = stats[:, 1:2]

            t0 = small.tile([P, 1], FP32)               # DVE: S*S
            nc.vector.tensor_mul(out=t0, in0=S, in1=S)
            e2 = small.tile([P, 1], FP32)               # DVE: SS - S²/cnt
            nc.vector.tensor_scalar(
                out=e2, in0=t0, scalar1=-1.0 / CNT, scalar2=SS,
                op0=mybir.AluOpType.mult, op1=mybir.AluOpType.add,
            )
            std = small.tile([P, 1], FP32)              # ACT: sqrt(E/(cnt-1)+eps)
            nc.scalar.activation(
                out=std, in_=e2, func=mybir.ActivationFunctionType.Sqrt,
                scale=1.0 / (CNT - 1), bias=eps_t,
            )
            rstd = small.tile([P, 1], FP32)             # DVE
            nc.vector.reciprocal(out=rstd, in_=std)
            nmean = small.tile([P, 1], FP32)            # ACT: -S/cnt (off path)
            nc.scalar.activation(
                out=nmean, in_=S, func=mybir.ActivationFunctionType.Copy,
                scale=-1.0 / CNT,
            )
            if affine:
                A_t = small.tile([P, 2], FP32)          # DVE: w * rstd
                nc.vector.tensor_scalar_mul(out=A_t, in0=w_t, scalar1=rstd)
                nmA = small.tile([P, 2], FP32)          # DVE: -mean * A
                nc.vector.tensor_scalar_mul(out=nmA, in0=A_t, scalar1=nmean)
                B_t = small.tile([P, 2], FP32)          # DVE: b - mean * A
                nc.vector.tensor_add(out=B_t, in0=b_t, in1=nmA)
            else:
                # weight == 1, bias == 0: A = rstd, B = -mean*rstd, identical
                # for both channel halves
                A_one = rstd
                B_one = small.tile([P, 1], FP32)        # DVE
                nc.vector.tensor_mul(out=B_one, in0=nmean, in1=rstd)

            # small chunks first: the first store DMA launches sooner
            norm_order = sorted(range(NCH), key=lambda ci: CHUNKS[ci][1] * CHUNKS[ci][3])
            for ci in norm_order:
                t0, nt, c0, clen = CHUNKS[ci]
                xt = chunk_tiles[ci]
                if affine:
                    for j in range(nt):
                        col = (t0 + j) % 2
                        nc.vector.tensor_scalar(
                            out=xt[:, j, :],
                            in0=xt[:, j, :],
                            scalar1=A_t[:, col : col + 1],
                            scalar2=B_t[:, col : col + 1],
                            op0=mybir.AluOpType.mult,
                            op1=mybir.AluOpType.add,
                        )
                else:
                    nc.vector.tensor_scalar(
                        out=xt.rearrange("p t f -> p (t f)"),
                        in0=xt.rearrange("p t f -> p (t f)"),
                        scalar1=A_one,
                        scalar2=B_one,
                        op0=mybir.AluOpType.mult,
                        op1=mybir.AluOpType.add,
                    )
                nc.sync.dma_start(
                    out=ov[:, t0 : t0 + nt, c0 : c0 + clen], in_=xt
                )

    nc.compile()
    return nc


_NC_CACHE: dict = {}


def _get_nc(affine: bool = True) -> bass.Bass:
    if affine not in _NC_CACHE:
        _NC_CACHE[affine] = build_nc(affine=affine)
    return _NC_CACHE[affine]


_RUNNER_CACHE: dict = {}


def _get_runner(nc: bass.Bass):
    """Like bass2jax.run_bass_via_pjrt, but inputs AND the donated zero
    output buffers are device_put + blocked BEFORE dispatch, so all 8 cores
    begin executing nearly simultaneously.  run_bass_via_pjrt passes host
    numpy arrays instead; the per-device H2D transfers then stagger the
    execution starts by tens of µs, which the NEFF entry barrier turns into
    dead time on every core."""
    import jax
    from jax.sharding import NamedSharding

    if id(nc) in _RUNNER_CACHE:
        return _RUNNER_CACHE[id(nc)]

    bass2jax.install_neuronx_cc_hook()
    partition_name = nc.partition_id_tensor.name if nc.partition_id_tensor else None

    in_names, out_names, out_avals = [], [], []
    for alloc in nc.m.functions[0].allocations:
        if not isinstance(alloc, mybir.MemoryLocationSet):
            continue
        name = alloc.memorylocations[0].name
        if alloc.kind == "ExternalInput":
            if name != partition_name:
                in_names.append(name)
        elif alloc.kind == "ExternalOutput":
            out_names.append(name)
            out_avals.append(
                jax.core.ShapedArray(
                    tuple(alloc.tensor_shape), mybir.dt.np(alloc.dtype)
                )
            )
    n_params = len(in_names)
    n_outs = len(out_names)
    all_in_names = list(in_names) + list(out_names)
    if partition_name is not None:
        all_in_names.append(partition_name)
    donate = tuple(range(n_params, n_params + n_outs))

    def _body(*args):
        operands = list(args)
        if partition_name is not None:
            operands.append(bass2jax.partition_id_tensor())
        outs = bass2jax._bass_exec_p.bind(
            *operands,
            out_avals=tuple(out_avals),
            in_names=tuple(all_in_names),
            out_names=tuple(out_names),
            lowering_input_output_aliases=(),
            sim_require_finite=True,
            sim_require_nnan=True,
            nc=nc,
        )
        return tuple(outs)

    devices = jax.devices()[:N_CORES]
    mesh = bass2jax.Mesh(np.asarray(devices), ("core",))
    in_specs = (bass2jax.PartitionSpec("core"),) * (n_params + n_outs)
    out_specs = (bass2jax.PartitionSpec("core"),) * n_outs
    sharded = jax.jit(
        bass2jax.shard_map(
            _body, mesh=mesh, in_specs=in_specs, out_specs=out_specs, check_rep=False
        ),
        donate_argnums=donate,
        keep_unused=True,
    )
    sharding = NamedSharding(mesh, bass2jax.PartitionSpec("core"))

    def run(in_maps):
        concat_in = [
            np.concatenate([np.asarray(in_maps[c][k]) for c in range(N_CORES)], axis=0)
            for k in in_names
        ]
        concat_zeros = [
            np.zeros((N_CORES * av.shape[0], *av.shape[1:]), av.dtype)
            for av in out_avals
        ]
        dev_args = [jax.device_put(a, sharding) for a in concat_in + concat_zeros]
        jax.block_until_ready(dev_args)
        out_arrs = sharded(*dev_args)
        out_arrs = jax.block_until_ready(out_arrs)
        return [
            {
                k: np.asarray(out_arrs[i]).reshape(N_CORES, *out_avals[i].shape)[c]
                for i, k in enumerate(out_names)
            }
            for c in range(N_CORES)
        ]

    _RUNNER_CACHE[id(nc)] = run
    return run


def kernel(x, weight, bias, indexes=None, **_unused):
    x = np.ascontiguousarray(np.asarray(x, dtype=np.float32))
    weight = np.ascontiguousarray(np.asarray(weight, dtype=np.float32).reshape(1, C, 1, 1))
    bias = np.ascontiguousarray(np.asarray(bias, dtype=np.float32).reshape(1, C, 1, 1))
    assert x.shape == (N, C, H, W)

    # the spec fills weight with ones and bias with zeros; when that holds the
    # specialized NEFF skips the per-channel coefficient path
    affine = not (np.all(weight == 1.0) and np.all(bias == 0.0))
    nc = _get_nc(affine)
    in_maps = []
    for i in range(N_CORES):
        m = {"x": np.ascontiguousarray(x[i * N_LOC : (i + 1) * N_LOC])}
        if affine:
            m["weight"] = weight
            m["bias"] = bias
        in_maps.append(m)
    try:
        results = _get_runner(nc)(in_maps)
    except Exception:
        # fall back to the stock SPMD runner (host-side numpy args; slightly
        # more core-start skew, but battle-tested)
        from concourse.bass_utils import run_bass_kernel_spmd

        results = run_bass_kernel_spmd(
            nc, in_maps, core_ids=list(range(N_CORES))
        ).results
    out = np.concatenate([results[i]["out"] for i in range(N_CORES)], axis=0)
    return out


if __name__ == "__main__":
    for aff in (False, True):
        nc = build_nc(affine=aff)
        print(f"build + compile OK (affine={aff}):", nc)

